# revision 20
# baseline (speedup 1.0000x reference)
"""Trainium2 Bass kernel: Llama-style attention prefill (B=2, S=2048, D=4096,
32 Q heads / 8 KV heads, head_dim 128, RoPE, additive mask), tensor-parallel
over heads across 8 NeuronCores.

Sharding (per core c):
  - Q heads 4c..4c+3 (wq columns c*512:(c+1)*512), KV head c (wk/wv columns
    c*128:(c+1)*128), wo column-shard wo[:, c*512:(c+1)*512].
  - Each core computes QKV projections + RoPE + attention for its heads,
    AllGathers the (transposed) attention outputs over all 8 cores (one AG
    per 512-token chunk, overlapped with compute), then computes a
    512-column slice of the output projection.
  - Host concatenates the 8 column slices -> full output.

Device-side layouts (all "T" = feature-on-partitions):
  - xT [4096 dm, 4096 tok] (tok = b*2048 + s), host-provided, bf16.
  - wq/wk columns are de-interleaved per head on the host: new col order
    [re pairs 0..63 | im pairs 0..63], so RoPE acts on partition halves.
    Scores are invariant (same permutation on Q and K); V/wo untouched.
  - Scores computed transposed: ST[k, t] = K @ Q^T; softmax over the
    partition axis k: exp (no max subtraction; |scores|*scale stays small
    so f32 exp is safe), then PV matmul with a ones-column appended to V
    producing both O[t, d] and the denominator L[t].
  - Causal diagonal 512-blocks are stair-stepped: k-sub-block j (128 wide)
    only needs q >= j*128, so its score matmul streams 512-128j columns
    into a packed PSUM region; the intra-128 triangular boundary is
    handled by multiplying e by a 0/1 triangular tile on the DVE.
  - O[t, d] tiles are transposed on the PE (identity-matmul transpose)
    into PSUM, copied to SBUF and DMA'd straight into the AllGather input
    (no DRAM staging roundtrip).
  - Queue roles: sync = loads only (x tiles, out-proj gathers), scalar =
    exp activations (+ final out writes), vector = DVE compute, gpsimd =
    resident loads + AG input writes + collective triggers. This keeps
    the exp stream clean and lets next-iteration x loads prefetch during
    attention instead of queueing behind attention-dependent stores.
"""
import numpy as np
import ml_dtypes

from concourse import bass, bacc, tile, mybir, bass_utils

F32 = mybir.dt.float32
BF16 = mybir.dt.bfloat16
Alu = mybir.AluOpType
Act = mybir.ActivationFunctionType

N_CORES = 8
B, S, D = 2, 2048, 4096
TOK = B * S                      # 4096 flattened tokens
HD = 128                         # head dim
HQ = 4                           # q heads per core
QW = HQ * HD                     # 512, per-core q width
SCALE = 1.0 / float(np.sqrt(HD))
NEG = -1e9

_BUILD_CACHE = {}


def _build(causal: bool):
    nc = bacc.Bacc("TRN2", target_bir_lowering=False, debug=False,
                   num_devices=N_CORES)
    # all inputs are pre-tiled on the host so every DMA is one contiguous
    # segment per partition (cheap descriptor generation)
    xT_d = nc.dram_tensor("xT", [8, 8, 128, 2048], BF16, kind="ExternalInput")
    # wq pre-split into column halves on the host so each half-load is one
    # contiguous segment per partition (cheap descriptor generation)
    wq_d = nc.dram_tensor("wq", [128, 2, 32, 256], BF16, kind="ExternalInput")
    wk_d = nc.dram_tensor("wk", [128, 32, HD], BF16, kind="ExternalInput")
    wv_d = nc.dram_tensor("wv", [128, 32, HD], BF16, kind="ExternalInput")
    wo_d = nc.dram_tensor("wo", [128, 32, QW], BF16, kind="ExternalInput")
    cosT_d = nc.dram_tensor("cosT", [64, S], BF16, kind="ExternalInput")
    sinT_d = nc.dram_tensor("sinT", [64, S], BF16, kind="ExternalInput")
    # 0/1 upper-triangular [k, q] 128-block mask + identity for PE transpose
    tri_d = nc.dram_tensor("tri01", [128, 128], BF16, kind="ExternalInput")
    idn_d = nc.dram_tensor("ident", [128, 128], BF16, kind="ExternalInput")
    if not causal:
        maskT_d = nc.dram_tensor("maskT", [S, S], BF16, kind="ExternalInput")
    out_d = nc.dram_tensor("out", [TOK, QW], F32, kind="ExternalOutput")

    with tile.TileContext(nc) as tc:
        with (
            tc.tile_pool(name="res", bufs=1) as res,            # residents
            tc.tile_pool(name="qtp", bufs=4 if causal else 8) as qtp,
            tc.tile_pool(name="ph2", bufs=2) as ph2,
            tc.tile_pool(name="epool", bufs=12) as epool,
            tc.tile_pool(name="mpool", bufs=9) as mpool,
            tc.tile_pool(name="onp", bufs=4) as onp,
            tc.tile_pool(name="agd", bufs=8, space="DRAM") as agd,
            tc.tile_pool(name="psum", bufs=2, space="PSUM") as psum,
        ):
            # ---- resident loads, in order of need. gpsimd DMAs go through
            # the slow software-descriptor path, so everything the startup
            # critical path needs rides sync/scalar (hardware path); only
            # wk before the first x block, the rest inside the first
            # proj_block, interleaved by deadline.
            wk_sb = res.tile([128, 32, HD], BF16, name="wk_sb")
            nc.sync.dma_start(wk_sb[:], wk_d[:])
            cos_sb = res.tile([64, S], BF16, name="cos_sb")
            sin_sb = res.tile([64, S], BF16, name="sin_sb")
            wv_sb = res.tile([128, 32, HD], BF16, name="wv_sb")
            tri_sb = res.tile([128, 128], BF16, name="tri_sb")
            idn_sb = res.tile([128, 128], BF16, name="idn_sb")

            # per-token-block K/V residents (block granularity keeps the
            # projection->attention dependency tracking per-block)
            kt_t = [res.tile([128, 512], BF16, name=f"kt{tb}")
                    for tb in range(8)]
            v_t = [res.tile([128, 4, 130], BF16, name=f"v{tb}")
                   for tb in range(8)]
            for tb in range(8):
                nc.vector.memset(v_t[tb][:, :, 128:129], 1.0)

            # ---------------- projections + RoPE for one 512-token block --
            def rope_store(ps, out_re, out_im, cos_sl, sin_sl, rp):
                t1 = rp.tile([64, 512], F32, name="t1", tag="t1", bufs=1)
                t2 = rp.tile([64, 512], F32, name="t2", tag="t2", bufs=1)
                nc.vector.tensor_mul(t1[:], ps[0:64, :], cos_sl)
                nc.vector.tensor_mul(t2[:], ps[64:128, :], sin_sl)
                nc.vector.tensor_sub(out_re, t1[:], t2[:])
                nc.vector.tensor_mul(t1[:], ps[0:64, :], sin_sl)
                nc.vector.tensor_mul(t2[:], ps[64:128, :], cos_sl)
                nc.vector.tensor_add(out_im, t1[:], t2[:])

            def proj_block(tb, wq_sb, xtp, ph1, first=False):
                # spread the very first block's loads across queues so the
                # PE can start within ~2us; later blocks ride the (load-
                # only, never attention-blocked) sync queue
                # 16 half-tiles per block, all on one tag: later blocks'
                # loads wait (WAR) on earlier blocks' last readers, which
                # self-throttles the startup HBM burst and gives a small
                # rolling prefetch cushion. The first block spreads across
                # the three DMA-capable queues for arrival-order latency.
                engs = ([nc.sync, nc.scalar] if first else [nc.sync])
                xts = []
                for gh in range(16):
                    xt = xtp.tile([128, 1024], BF16, name="xt", tag="xt",
                                  bufs=20)
                    engs[gh % len(engs)].dma_start(
                        xt[:],
                        xT_d[tb, gh // 2, :,
                             (gh % 2) * 1024:(gh % 2 + 1) * 1024])
                    xts.append(xt)
                if first:
                    # deferred residents + wq, by deadline (rope -> V -> Q
                    # -> attn), behind the first block's x triggers
                    nc.scalar.dma_start(cos_sb[:], cosT_d[:])
                    nc.scalar.dma_start(sin_sb[:], sinT_d[:])
                    nc.scalar.dma_start(wv_sb[:], wv_d[:])
                    nc.scalar.dma_start(wq_sb[0][:], wq_d[:, 0])
                    nc.scalar.dma_start(wq_sb[1][:], wq_d[:, 1])
                    nc.gpsimd.dma_start(tri_sb[:], tri_d[:])
                    nc.gpsimd.dma_start(idn_sb[:], idn_d[:])

                def xsl(ic):
                    return xts[ic // 2][:,
                                        (ic % 2) * 512:(ic % 2 + 1) * 512]

                pos = (tb % 4) * 512
                cos_sl = cos_sb[:, pos:pos + 512]
                sin_sl = sin_sb[:, pos:pos + 512]

                ps_k = psum.tile([128, 512], F32, name="ps_k", tag="ps")
                for ic in range(32):
                    nc.tensor.matmul(ps_k[:], wk_sb[:, ic, :], xsl(ic),
                                     start=(ic == 0), stop=(ic == 31))
                rope_store(ps_k, kt_t[tb][0:64, :], kt_t[tb][64:128, :],
                           cos_sl, sin_sl, ph1)

                # V in natural [token, d] layout, computed directly:
                # lhsT = xT chunk [i, t(128)], rhs = wv chunk [i, d]
                for j in range(4):
                    ps_v = psum.tile([128, 128], F32, name="ps_v", tag="ps")
                    for ic in range(32):
                        nc.tensor.matmul(
                            ps_v[:],
                            xsl(ic)[:, j * 128:(j + 1) * 128],
                            wv_sb[:, ic, :],
                            start=(ic == 0), stop=(ic == 31))
                    nc.vector.tensor_copy(v_t[tb][:, j, 0:128], ps_v[:])

                qt = qtp.tile([128, HQ, 512], BF16, name="qt", tag="qt")
                for dq in range(HQ):
                    ps_q = psum.tile([128, 512], F32, name="ps_q", tag="ps")
                    for ic in range(32):
                        nc.tensor.matmul(
                            ps_q[:],
                            wq_sb[dq // 2][:, ic,
                                           (dq % 2) * HD:(dq % 2 + 1) * HD],
                            xsl(ic),
                            start=(ic == 0), stop=(ic == 31))
                    rope_store(ps_q, qt[0:64, dq, :], qt[64:128, dq, :],
                               cos_sl, sin_sl, ph1)
                return qt

            # three AllGathers (A: qb0-1 chunks, B: qb2, C: qb3), sized so
            # AG-A/B trigger mid-kernel and AG-C hides under the out-
            # projections of A+B
            GRP_SLOTS = [2, 2, 2, 2]
            ag_in_g = [agd.tile([512, 512 * n], BF16, name=f"ag_in{g}")
                       for g, n in enumerate(GRP_SLOTS)]
            ag_out_g = [agd.tile([D, 512 * n], BF16, name=f"ag_out{g}",
                                 addr_space="Shared")
                        for g, n in enumerate(GRP_SLOTS)]

            def chunk_group(qb, b):
                return qb, b

            # pending head-output transposes: (o_list, g, slot, hh).
            # Transposing o[t,d] tiles happens on the PE one head-group
            # late so the PE never waits on the DVE normalize that
            # produces them; stragglers flush before the AllGather.
            pending = []

            def emit_ot(o_list, g, slot, hh):
                # transpose via a normal-mode matmul against the identity
                # (out[d,t'] = sum_t o[t,d] I[t,t']): the weight load is the
                # ordinary overlappable LDWEIGHTS, unlike is_transpose mode
                otp = psum.tile([128, 512], F32, name="otp", tag="o")
                for ts in range(4):
                    nc.tensor.matmul(otp[:, ts * 128:(ts + 1) * 128],
                                     o_list[ts][:], idn_sb[:],
                                     start=True, stop=True)
                ag_sb = onp.tile([128, 512], BF16, name="ag_sb", tag="ag",
                                 bufs=3)
                nc.vector.tensor_copy(ag_sb[:], otp[:])
                nc.gpsimd.dma_start(
                    ag_in_g[g][hh * 128:(hh + 1) * 128,
                               slot * 512:(slot + 1) * 512],
                    ag_sb[:])

            def flush_pending():
                while pending:
                    emit_ot(*pending.pop(0))

            # ---------------- attention for one 512-token chunk -----------
            def attn_chunk(qb, b, qt, mtiles, kcs):
                g, slot = chunk_group(qb, b)
                pairs = [(kcs[i], kcs[i + 1]) for i in range(0, len(kcs), 2)]
                for h in range(HQ):
                    e_full = {}
                    for pr in pairs:
                        # two score tiles in one 2-bank PSUM tile -> one
                        # exp over [128, 1024] (halves ACT op count)
                        st_ps = psum.tile([128, 1024], F32, name="st_ps",
                                          tag="st")
                        for j, kc in enumerate(pr):
                            tbk = b * 4 + kc // 4
                            kof = (kc % 4) * 128
                            nc.tensor.matmul(
                                st_ps[:, j * 512:(j + 1) * 512],
                                kt_t[tbk][:, kof:kof + 128],
                                qt[:, h, :],
                                start=True, stop=True)
                        e_t = epool.tile([128, 1024], BF16, name="e_t",
                                         tag="e")
                        if mtiles.get(pr) is not None:
                            pre = ph2.tile([128, 1024], F32, name="pre",
                                           tag="pre", bufs=1)
                            nc.vector.scalar_tensor_tensor(
                                pre[:], st_ps[:], SCALE, mtiles[pr],
                                Alu.mult, Alu.add)
                            nc.scalar.activation(e_t[:], pre[:], Act.Exp)
                        else:
                            nc.scalar.activation(e_t[:], st_ps[:], Act.Exp,
                                                 scale=SCALE)
                        e_full[pr] = e_t
                    if causal:
                        # stair-stepped diagonal: sub-block j only needs
                        # q >= j*128. Packed: A = j0 [0:512] | j1 [512:896],
                        # B = j2 [0:256] | j3 [256:384]. The leading 128
                        # cols of each j are triangular -> multiply by the
                        # 0/1 tile after exp.
                        tbq = b * 4 + qb
                        stA = psum.tile([128, 1024], F32, name="st_ps",
                                        tag="st")
                        nc.tensor.matmul(stA[:, 0:512],
                                         kt_t[tbq][:, 0:128],
                                         qt[:, h, :], start=True, stop=True)
                        nc.tensor.matmul(stA[:, 512:896],
                                         kt_t[tbq][:, 128:256],
                                         qt[:, h, 128:512],
                                         start=True, stop=True)
                        eA = epool.tile([128, 1024], BF16, name="e_t",
                                        tag="e")
                        nc.scalar.activation(eA[:, 0:896], stA[:, 0:896],
                                             Act.Exp, scale=SCALE)
                        nc.vector.tensor_mul(eA[:, 0:128], eA[:, 0:128],
                                             tri_sb[:])
                        nc.vector.tensor_mul(eA[:, 512:640],
                                             eA[:, 512:640], tri_sb[:])
                        stB = psum.tile([128, 1024], F32, name="st_ps",
                                        tag="st")
                        nc.tensor.matmul(stB[:, 0:256],
                                         kt_t[tbq][:, 256:384],
                                         qt[:, h, 256:512],
                                         start=True, stop=True)
                        nc.tensor.matmul(stB[:, 256:384],
                                         kt_t[tbq][:, 384:512],
                                         qt[:, h, 384:512],
                                         start=True, stop=True)
                        eB = epool.tile([128, 1024], BF16, name="e_t",
                                        tag="e")
                        nc.scalar.activation(eB[:, 0:384], stB[:, 0:384],
                                             Act.Exp, scale=SCALE)
                        nc.vector.tensor_mul(eB[:, 0:128], eB[:, 0:128],
                                             tri_sb[:])
                        nc.vector.tensor_mul(eB[:, 256:384],
                                             eB[:, 256:384], tri_sb[:])
                    o_list = []
                    for ts in range(4):
                        o_ps = psum.tile([128, 129], F32, name="o_ps",
                                         tag="o")
                        contribs = []
                        for pr in pairs:
                            for j, kc in enumerate(pr):
                                contribs.append(
                                    (e_full[pr][:, j * 512 + ts * 128:
                                                j * 512 + (ts + 1) * 128],
                                     kc))
                        if causal:
                            kd = 4 * qb
                            contribs.append(
                                (eA[:, ts * 128:(ts + 1) * 128], kd))
                            if ts >= 1:
                                contribs.append(
                                    (eA[:, 512 + (ts - 1) * 128:
                                         512 + ts * 128], kd + 1))
                            if ts >= 2:
                                contribs.append(
                                    (eB[:, (ts - 2) * 128:(ts - 1) * 128],
                                     kd + 2))
                            if ts >= 3:
                                contribs.append((eB[:, 256:384], kd + 3))
                        for i, (esl, kc) in enumerate(contribs):
                            nc.tensor.matmul(
                                o_ps[:], esl,
                                v_t[b * 4 + kc // 4][:, kc % 4, 0:129],
                                start=(i == 0),
                                stop=(i == len(contribs) - 1))
                        linv = onp.tile([128, 1], F32, name="linv",
                                        tag="linv", bufs=4)
                        nc.vector.reciprocal(linv[:], o_ps[:, 128:129])
                        o_n = onp.tile([128, 128], BF16, name="o_n",
                                       tag="o_n", bufs=12)
                        nc.vector.tensor_scalar(
                            o_n[:], o_ps[:, 0:128], linv[:], None, Alu.mult)
                        o_list.append(o_n)
                    # transpose the PREVIOUS head-group's outputs now: its
                    # DVE normalizes finished while this group's scores ran
                    pending.append((o_list, g, slot, h))
                    if len(pending) > 1:
                        emit_ot(*pending.pop(0))

            def ag_group(g):
                flush_pending()
                nc.gpsimd.collective_compute(
                    "AllGather", Alu.bypass,
                    replica_groups=[list(range(N_CORES))],
                    ins=[ag_in_g[g][:]], outs=[ag_out_g[g][:]])

            # ---------------- emission ------------------------------------
            def make_mtiles(qb):
                if causal:
                    # full blocks only; the diagonal is stair-stepped inline
                    return {}, list(range(4 * qb))
                kcs = list(range(16))
                mtiles = {}
                for kc in range(0, 16, 2):
                    mt = mpool.tile([128, 2, 512], BF16, name="mt",
                                    tag="mt")
                    nc.sync.dma_start(
                        mt[:],
                        maskT_d[kc * 128:(kc + 2) * 128,
                                qb * 512:(qb + 1) * 512]
                        .rearrange("(two p) t -> p two t", p=128))
                    mtiles[(kc, kc + 1)] = mt[:]
                return mtiles, kcs

            with (
                tc.tile_pool(name="wqp", bufs=1) as wqp,
                # bufs=8 = one block: the next block's loads queue behind
                # this block's last readers, which both self-throttles the
                # startup HBM burst and prefetches during attention
                tc.tile_pool(name="xtp", bufs=8) as xtp,
                tc.tile_pool(name="ph1", bufs=2) as ph1,
            ):
                # wq split in two column-half tiles: the Q chains need the
                # first half well before the second, and dependency
                # tracking is tile-granular (loads emitted in proj_block)
                wq_sb = [wqp.tile([128, 32, 256], BF16, name=f"wq_sb{i}")
                         for i in range(2)]
                # tiny dummy AllGather emitted after the weight loads:
                # aligns the cores' start skew while the first projection
                # blocks keep the PE busy, so AG0 isn't delayed by launch
                # offset
                dummy_in = agd.tile([64, 4], BF16, name="dummy_in")
                nc.gpsimd.dma_start(dummy_in[:], cosT_d[0:64, 0:4])
                dummy_out = agd.tile([512, 4], BF16, name="dummy_out",
                                     addr_space="Shared")
                nc.gpsimd.collective_compute(
                    "AllGather", Alu.bypass,
                    replica_groups=[list(range(N_CORES))],
                    ins=[dummy_in[:]], outs=[dummy_out[:]])
                if causal:
                    # causal: chunk (qb, b) needs only blocks <= qb, so
                    # projections and attention interleave per qb
                    for qb in range(4):
                        qts = [proj_block(b * 4 + qb, wq_sb, xtp, ph1,
                                          first=(qb == 0 and b == 0))
                               for b in range(B)]
                        mtiles, kcs = make_mtiles(qb)
                        if qb < 3:
                            for b in range(B):
                                attn_chunk(qb, b, qts[b], mtiles, kcs)
                            ag_group(qb)
                        else:
                            qts3, mtiles3, kcs3 = qts, mtiles, kcs
                else:
                    # general mask: every chunk may attend to every block,
                    # so all projections must complete first
                    allq = [[None, None] for _ in range(4)]
                    for qb in range(4):
                        for b in range(B):
                            allq[qb][b] = proj_block(
                                b * 4 + qb, wq_sb, xtp, ph1,
                                first=(qb == 0 and b == 0))
                    for qb in range(3):
                        mtiles, kcs = make_mtiles(qb)
                        for b in range(B):
                            attn_chunk(qb, b, allq[qb][b], mtiles, kcs)
                        ag_group(qb)
                    qts3 = allq[3]
                    mtiles3, kcs3 = make_mtiles(3)

            # final phase: wo reuses wq's SBUF lifetime; the last two
            # attention chunks are pipelined against the first two
            # out-projections, so every out-proj's transposing og load gates
            # only on already-completed AllGathers
            with (
                tc.tile_pool(name="wop", bufs=1) as wop,
                tc.tile_pool(name="ogp", bufs=2) as ogp,
            ):
                wo_sb = wop.tile([128, 32, QW], BF16, name="wo_sb")
                nc.gpsimd.dma_start(wo_sb[:], wo_d[:])

                def out_proj(ag_out, slot, t0):
                    # one gather per chunk: same descriptor count as two
                    # half loads but double the segment size
                    og_sb = ogp.tile([128, 32, 512], BF16, name="og_sb",
                                     tag="og")
                    nc.sync.dma_start(
                        og_sb[:],
                        ag_out[:, slot * 512:(slot + 1) * 512]
                        .rearrange("(hc p) t -> p hc t", p=128))
                    for tc2 in range(4):
                        op_ps = psum.tile([128, 512], F32, name="op_ps",
                                          tag="ps")
                        for hc in range(32):
                            nc.tensor.matmul(
                                op_ps[:],
                                og_sb[:, hc, tc2 * 128:(tc2 + 1) * 128],
                                wo_sb[:, hc, :],
                                start=(hc == 0), stop=(hc == 31))
                        oo = ph2.tile([128, 512], F32, name="oo",
                                      tag="oo")
                        nc.vector.tensor_copy(oo[:], op_ps[:])
                        nc.scalar.dma_start(
                            out_d[t0 + tc2 * 128: t0 + (tc2 + 1) * 128, :],
                            oo[:])

                for b3 in range(B):
                    attn_chunk(3, b3, qts3[b3], mtiles3, kcs3)
                ag_group(3)
                for qb in range(4):
                    for b in range(B):
                        g, slot = chunk_group(qb, b)
                        out_proj(ag_out_g[g], slot, b * S + qb * 512)

    nc.compile()
    return nc


def _get_nc(causal: bool):
    if causal not in _BUILD_CACHE:
        _BUILD_CACHE[causal] = _build(causal)
    return _BUILD_CACHE[causal]


_DEINT = np.concatenate([np.arange(0, HD, 2), np.arange(1, HD, 2)])


def _deinterleave(w):
    """Permute per-head columns [0,2,..,126,1,3,..,127] (re block | im block)."""
    out = w.copy()
    nh = w.shape[1] // HD
    for h in range(nh):
        out[:, h * HD:(h + 1) * HD] = w[:, h * HD + _DEINT]
    return out


def make_in_maps(x, wq, wk, wv, wo, freqs_cos, freqs_sin, mask, causal):
    bf = ml_dtypes.bfloat16

    def tile_w(w):
        # [4096, W] -> [128, 32, W]: out[p, ic, d] = w[ic*128+p, d]
        return np.ascontiguousarray(
            w.reshape(32, 128, -1).transpose(1, 0, 2)).astype(bf)

    xT = np.asarray(x, np.float32).reshape(TOK, D).T        # [dm, tok]
    # [tb, g, p, ic_in*512+t] = xT[g*512+ic_in*128+p, tb*512+t]
    xt_host = np.ascontiguousarray(
        xT.reshape(8, 4, 128, 8, 512).transpose(3, 0, 2, 1, 4)
        .reshape(8, 8, 128, 2048)).astype(bf)
    cosT = np.ascontiguousarray(np.asarray(freqs_cos, np.float32).T).astype(bf)
    sinT = np.ascontiguousarray(np.asarray(freqs_sin, np.float32).T).astype(bf)
    mask = np.asarray(mask, np.float32)
    # tri01[k, t] = 1 where t >= k (valid) within a diagonal 128-block
    tri01 = np.triu(np.ones((128, 128), np.float32)).astype(bf)
    ident = np.eye(128, dtype=np.float32).astype(bf)

    in_maps = []
    for c in range(N_CORES):
        wqc = tile_w(_deinterleave(
            np.asarray(wq[:, c * QW:(c + 1) * QW], np.float32)))
        m = {
            "xT": xt_host,
            # [128, 32, 512] -> [128, 2, 32, 256] (column halves major)
            "wq": np.ascontiguousarray(
                wqc.reshape(128, 32, 2, 256).transpose(0, 2, 1, 3)),
            "wk": tile_w(_deinterleave(
                np.asarray(wk[:, c * HD:(c + 1) * HD], np.float32))),
            "wv": tile_w(np.asarray(wv[:, c * HD:(c + 1) * HD], np.float32)),
            "wo": tile_w(np.asarray(wo[:, c * QW:(c + 1) * QW], np.float32)),
            "cosT": cosT,
            "sinT": sinT,
            "tri01": tri01,
            "ident": ident,
        }
        if not causal:
            m["maskT"] = np.ascontiguousarray(mask.T).astype(bf)
        in_maps.append(m)
    return in_maps


def _is_causal(mask):
    mask = np.asarray(mask, np.float32)
    expect = np.where(np.tril(np.ones((S, S), bool)), 0.0, NEG).astype(np.float32)
    return np.array_equal(mask, expect)


def kernel(x, wq, wk, wv, wo, cache_k, cache_v, freqs_cos, freqs_sin, mask,
           start_pos):
    assert int(start_pos) == 0, "kernel hardcodes start_pos=0 prefill"
    assert tuple(np.shape(x)) == (B, S, D)
    causal = _is_causal(mask)
    nc = _get_nc(causal)
    in_maps = make_in_maps(x, wq, wk, wv, wo, freqs_cos, freqs_sin, mask,
                           causal)
    res = bass_utils.run_bass_kernel_spmd(
        nc, in_maps, core_ids=list(range(N_CORES)))
    out = np.empty((TOK, D), np.float32)
    for c in range(N_CORES):
        out[:, c * QW:(c + 1) * QW] = res.results[c]["out"]
    return out.reshape(B, S, D)


# revision 21
# speedup vs baseline: 1.0174x; 1.0174x over previous
"""Trainium2 Bass kernel: Llama-style attention prefill (B=2, S=2048, D=4096,
32 Q heads / 8 KV heads, head_dim 128, RoPE, additive mask), tensor-parallel
over heads across 8 NeuronCores.

Sharding (per core c):
  - Q heads 4c..4c+3 (wq columns c*512:(c+1)*512), KV head c (wk/wv columns
    c*128:(c+1)*128), wo column-shard wo[:, c*512:(c+1)*512].
  - Each core computes QKV projections + RoPE + attention for its heads,
    AllGathers the (transposed) attention outputs over all 8 cores (one AG
    per 512-token chunk, overlapped with compute), then computes a
    512-column slice of the output projection.
  - Host concatenates the 8 column slices -> full output.

Device-side layouts (all "T" = feature-on-partitions):
  - xT [4096 dm, 4096 tok] (tok = b*2048 + s), host-provided, bf16.
  - wq/wk columns are de-interleaved per head on the host: new col order
    [re pairs 0..63 | im pairs 0..63], so RoPE acts on partition halves.
    Scores are invariant (same permutation on Q and K); V/wo untouched.
  - Scores computed transposed: ST[k, t] = K @ Q^T; softmax over the
    partition axis k: exp (no max subtraction; |scores|*scale stays small
    so f32 exp is safe), then PV matmul with a ones-column appended to V
    producing both O[t, d] and the denominator L[t].
  - Causal diagonal 512-blocks are stair-stepped: k-sub-block j (128 wide)
    only needs q >= j*128, so its score matmul streams 512-128j columns
    into a packed PSUM region; the intra-128 triangular boundary is
    handled by multiplying e by a 0/1 triangular tile on the DVE.
  - O[t, d] tiles are transposed on the PE (identity-matmul transpose)
    into PSUM, copied to SBUF and DMA'd straight into the AllGather input
    (no DRAM staging roundtrip).
  - Queue roles: sync = loads only (x tiles, out-proj gathers), scalar =
    exp activations (+ final out writes), vector = DVE compute, gpsimd =
    resident loads + AG input writes + collective triggers. This keeps
    the exp stream clean and lets next-iteration x loads prefetch during
    attention instead of queueing behind attention-dependent stores.
"""
import numpy as np
import ml_dtypes

from concourse import bass, bacc, tile, mybir, bass_utils

F32 = mybir.dt.float32
BF16 = mybir.dt.bfloat16
Alu = mybir.AluOpType
Act = mybir.ActivationFunctionType

N_CORES = 8
B, S, D = 2, 2048, 4096
TOK = B * S                      # 4096 flattened tokens
HD = 128                         # head dim
HQ = 4                           # q heads per core
QW = HQ * HD                     # 512, per-core q width
SCALE = 1.0 / float(np.sqrt(HD))
NEG = -1e9

_BUILD_CACHE = {}


def _build(causal: bool):
    nc = bacc.Bacc("TRN2", target_bir_lowering=False, debug=False,
                   num_devices=N_CORES)
    # all inputs are pre-tiled on the host so every DMA is one contiguous
    # segment per partition (cheap descriptor generation)
    xT_d = nc.dram_tensor("xT", [8, 8, 128, 2048], BF16, kind="ExternalInput")
    # wq pre-split into column halves on the host so each half-load is one
    # contiguous segment per partition (cheap descriptor generation)
    wq_d = nc.dram_tensor("wq", [128, 2, 32, 256], BF16, kind="ExternalInput")
    wk_d = nc.dram_tensor("wk", [128, 32, HD], BF16, kind="ExternalInput")
    wv_d = nc.dram_tensor("wv", [128, 32, HD], BF16, kind="ExternalInput")
    wo_d = nc.dram_tensor("wo", [128, 32, QW], BF16, kind="ExternalInput")
    cosT_d = nc.dram_tensor("cosT", [64, S], BF16, kind="ExternalInput")
    sinT_d = nc.dram_tensor("sinT", [64, S], BF16, kind="ExternalInput")
    # 0/1 upper-triangular [k, q] 128-block mask + identity for PE transpose
    tri_d = nc.dram_tensor("tri01", [128, 128], BF16, kind="ExternalInput")
    idn_d = nc.dram_tensor("ident", [128, 128], BF16, kind="ExternalInput")
    if not causal:
        maskT_d = nc.dram_tensor("maskT", [S, S], BF16, kind="ExternalInput")
    out_d = nc.dram_tensor("out", [TOK, QW], F32, kind="ExternalOutput")

    with tile.TileContext(nc) as tc:
        with (
            tc.tile_pool(name="res", bufs=1) as res,            # residents
            tc.tile_pool(name="qtp", bufs=4 if causal else 8) as qtp,
            tc.tile_pool(name="ph2", bufs=2) as ph2,
            tc.tile_pool(name="epool", bufs=12) as epool,
            tc.tile_pool(name="mpool", bufs=9) as mpool,
            tc.tile_pool(name="onp", bufs=4) as onp,
            tc.tile_pool(name="agd", bufs=8, space="DRAM") as agd,
            tc.tile_pool(name="psum", bufs=2, space="PSUM") as psum,
        ):
            # ---- resident loads, in order of need. gpsimd DMAs go through
            # the slow software-descriptor path, so everything the startup
            # critical path needs rides sync/scalar (hardware path); only
            # wk before the first x block, the rest inside the first
            # proj_block, interleaved by deadline.
            wk_sb = res.tile([128, 32, HD], BF16, name="wk_sb")
            nc.sync.dma_start(wk_sb[:], wk_d[:])
            cos_sb = res.tile([64, S], BF16, name="cos_sb")
            sin_sb = res.tile([64, S], BF16, name="sin_sb")
            wv_sb = res.tile([128, 32, HD], BF16, name="wv_sb")
            tri_sb = res.tile([128, 128], BF16, name="tri_sb")
            idn_sb = res.tile([128, 128], BF16, name="idn_sb")

            # per-token-block K/V residents (block granularity keeps the
            # projection->attention dependency tracking per-block)
            kt_t = [res.tile([128, 512], BF16, name=f"kt{tb}")
                    for tb in range(8)]
            v_t = [res.tile([128, 4, 130], BF16, name=f"v{tb}")
                   for tb in range(8)]
            for tb in range(8):
                nc.vector.memset(v_t[tb][:, :, 128:129], 1.0)

            # ---------------- projections + RoPE for one 512-token block --
            def rope_store(ps, out_re, out_im, cos_sl, sin_sl, rp):
                t1 = rp.tile([64, 512], F32, name="t1", tag="t1", bufs=1)
                t2 = rp.tile([64, 512], F32, name="t2", tag="t2", bufs=1)
                nc.vector.tensor_mul(t1[:], ps[0:64, :], cos_sl)
                nc.vector.tensor_mul(t2[:], ps[64:128, :], sin_sl)
                nc.vector.tensor_sub(out_re, t1[:], t2[:])
                nc.vector.tensor_mul(t1[:], ps[0:64, :], sin_sl)
                nc.vector.tensor_mul(t2[:], ps[64:128, :], cos_sl)
                nc.vector.tensor_add(out_im, t1[:], t2[:])

            def proj_block(tb, wq_sb, xtp, ph1, first=False):
                # spread the very first block's loads across queues so the
                # PE can start within ~2us; later blocks ride the (load-
                # only, never attention-blocked) sync queue
                # 16 half-tiles per block, all on one tag: later blocks'
                # loads wait (WAR) on earlier blocks' last readers, which
                # self-throttles the startup HBM burst and gives a small
                # rolling prefetch cushion. The first block spreads across
                # the three DMA-capable queues for arrival-order latency.
                engs = ([nc.sync, nc.scalar] if first else [nc.sync])
                xts = []
                for gh in range(16):
                    xt = xtp.tile([128, 1024], BF16, name="xt", tag="xt",
                                  bufs=32)
                    engs[gh % len(engs)].dma_start(
                        xt[:],
                        xT_d[tb, gh // 2, :,
                             (gh % 2) * 1024:(gh % 2 + 1) * 1024])
                    xts.append(xt)
                if first:
                    # deferred residents + wq, by deadline (rope -> V -> Q
                    # -> attn), behind the first block's x triggers. wq1 on
                    # sync ahead of the next blocks' loads: in-order DMA
                    # within the channel protects its deadline from them.
                    nc.scalar.dma_start(cos_sb[:], cosT_d[:])
                    nc.scalar.dma_start(sin_sb[:], sinT_d[:])
                    nc.scalar.dma_start(wv_sb[:], wv_d[:])
                    nc.scalar.dma_start(wq_sb[0][:], wq_d[:, 0])
                    nc.sync.dma_start(wq_sb[1][:], wq_d[:, 1])
                    nc.gpsimd.dma_start(tri_sb[:], tri_d[:])
                    nc.gpsimd.dma_start(idn_sb[:], idn_d[:])

                def xsl(ic):
                    return xts[ic // 2][:,
                                        (ic % 2) * 512:(ic % 2 + 1) * 512]

                pos = (tb % 4) * 512
                cos_sl = cos_sb[:, pos:pos + 512]
                sin_sl = sin_sb[:, pos:pos + 512]

                ps_k = psum.tile([128, 512], F32, name="ps_k", tag="ps")
                for ic in range(32):
                    nc.tensor.matmul(ps_k[:], wk_sb[:, ic, :], xsl(ic),
                                     start=(ic == 0), stop=(ic == 31))
                rope_store(ps_k, kt_t[tb][0:64, :], kt_t[tb][64:128, :],
                           cos_sl, sin_sl, ph1)

                # V in natural [token, d] layout, computed directly:
                # lhsT = xT chunk [i, t(128)], rhs = wv chunk [i, d]
                for j in range(4):
                    ps_v = psum.tile([128, 128], F32, name="ps_v", tag="ps")
                    for ic in range(32):
                        nc.tensor.matmul(
                            ps_v[:],
                            xsl(ic)[:, j * 128:(j + 1) * 128],
                            wv_sb[:, ic, :],
                            start=(ic == 0), stop=(ic == 31))
                    nc.vector.tensor_copy(v_t[tb][:, j, 0:128], ps_v[:])

                qt = qtp.tile([128, HQ, 512], BF16, name="qt", tag="qt")
                for dq in range(HQ):
                    ps_q = psum.tile([128, 512], F32, name="ps_q", tag="ps")
                    for ic in range(32):
                        nc.tensor.matmul(
                            ps_q[:],
                            wq_sb[dq // 2][:, ic,
                                           (dq % 2) * HD:(dq % 2 + 1) * HD],
                            xsl(ic),
                            start=(ic == 0), stop=(ic == 31))
                    rope_store(ps_q, qt[0:64, dq, :], qt[64:128, dq, :],
                               cos_sl, sin_sl, ph1)
                return qt

            # three AllGathers (A: qb0-1 chunks, B: qb2, C: qb3), sized so
            # AG-A/B trigger mid-kernel and AG-C hides under the out-
            # projections of A+B
            GRP_SLOTS = [2, 2, 2, 2]
            ag_in_g = [agd.tile([512, 512 * n], BF16, name=f"ag_in{g}")
                       for g, n in enumerate(GRP_SLOTS)]
            ag_out_g = [agd.tile([D, 512 * n], BF16, name=f"ag_out{g}",
                                 addr_space="Shared")
                        for g, n in enumerate(GRP_SLOTS)]

            def chunk_group(qb, b):
                return qb, b

            # pending head-output transposes: (o_list, g, slot, hh).
            # Transposing o[t,d] tiles happens on the PE one head-group
            # late so the PE never waits on the DVE normalize that
            # produces them; stragglers flush before the AllGather.
            pending = []

            def emit_ot(o_list, g, slot, hh):
                # transpose via a normal-mode matmul against the identity
                # (out[d,t'] = sum_t o[t,d] I[t,t']): the weight load is the
                # ordinary overlappable LDWEIGHTS, unlike is_transpose mode
                otp = psum.tile([128, 512], F32, name="otp", tag="o")
                for ts in range(4):
                    nc.tensor.matmul(otp[:, ts * 128:(ts + 1) * 128],
                                     o_list[ts][:], idn_sb[:],
                                     start=True, stop=True)
                ag_sb = onp.tile([128, 512], BF16, name="ag_sb", tag="ag",
                                 bufs=3)
                nc.vector.tensor_copy(ag_sb[:], otp[:])
                nc.gpsimd.dma_start(
                    ag_in_g[g][hh * 128:(hh + 1) * 128,
                               slot * 512:(slot + 1) * 512],
                    ag_sb[:])

            def flush_pending():
                while pending:
                    emit_ot(*pending.pop(0))

            # ---------------- attention for one 512-token chunk -----------
            def attn_chunk(qb, b, qt, mtiles, kcs):
                g, slot = chunk_group(qb, b)
                pairs = [(kcs[i], kcs[i + 1]) for i in range(0, len(kcs), 2)]
                for h in range(HQ):
                    e_full = {}
                    for pr in pairs:
                        # two score tiles in one 2-bank PSUM tile -> one
                        # exp over [128, 1024] (halves ACT op count)
                        st_ps = psum.tile([128, 1024], F32, name="st_ps",
                                          tag="st")
                        for j, kc in enumerate(pr):
                            tbk = b * 4 + kc // 4
                            kof = (kc % 4) * 128
                            nc.tensor.matmul(
                                st_ps[:, j * 512:(j + 1) * 512],
                                kt_t[tbk][:, kof:kof + 128],
                                qt[:, h, :],
                                start=True, stop=True)
                        e_t = epool.tile([128, 1024], BF16, name="e_t",
                                         tag="e")
                        if mtiles.get(pr) is not None:
                            pre = ph2.tile([128, 1024], F32, name="pre",
                                           tag="pre", bufs=1)
                            nc.vector.scalar_tensor_tensor(
                                pre[:], st_ps[:], SCALE, mtiles[pr],
                                Alu.mult, Alu.add)
                            nc.scalar.activation(e_t[:], pre[:], Act.Exp)
                        else:
                            nc.scalar.activation(e_t[:], st_ps[:], Act.Exp,
                                                 scale=SCALE)
                        e_full[pr] = e_t
                    if causal:
                        # stair-stepped diagonal: sub-block j only needs
                        # q >= j*128. Packed: A = j0 [0:512] | j1 [512:896],
                        # B = j2 [0:256] | j3 [256:384]. The leading 128
                        # cols of each j are triangular -> multiply by the
                        # 0/1 tile after exp.
                        tbq = b * 4 + qb
                        stA = psum.tile([128, 1024], F32, name="st_ps",
                                        tag="st")
                        nc.tensor.matmul(stA[:, 0:512],
                                         kt_t[tbq][:, 0:128],
                                         qt[:, h, :], start=True, stop=True)
                        nc.tensor.matmul(stA[:, 512:896],
                                         kt_t[tbq][:, 128:256],
                                         qt[:, h, 128:512],
                                         start=True, stop=True)
                        eA = epool.tile([128, 1024], BF16, name="e_t",
                                        tag="e")
                        nc.scalar.activation(eA[:, 0:896], stA[:, 0:896],
                                             Act.Exp, scale=SCALE)
                        nc.vector.tensor_mul(eA[:, 0:128], eA[:, 0:128],
                                             tri_sb[:])
                        nc.vector.tensor_mul(eA[:, 512:640],
                                             eA[:, 512:640], tri_sb[:])
                        stB = psum.tile([128, 1024], F32, name="st_ps",
                                        tag="st")
                        nc.tensor.matmul(stB[:, 0:256],
                                         kt_t[tbq][:, 256:384],
                                         qt[:, h, 256:512],
                                         start=True, stop=True)
                        nc.tensor.matmul(stB[:, 256:384],
                                         kt_t[tbq][:, 384:512],
                                         qt[:, h, 384:512],
                                         start=True, stop=True)
                        eB = epool.tile([128, 1024], BF16, name="e_t",
                                        tag="e")
                        nc.scalar.activation(eB[:, 0:384], stB[:, 0:384],
                                             Act.Exp, scale=SCALE)
                        nc.vector.tensor_mul(eB[:, 0:128], eB[:, 0:128],
                                             tri_sb[:])
                        nc.vector.tensor_mul(eB[:, 256:384],
                                             eB[:, 256:384], tri_sb[:])
                    o_list = []
                    for ts in range(4):
                        o_ps = psum.tile([128, 129], F32, name="o_ps",
                                         tag="o")
                        contribs = []
                        for pr in pairs:
                            for j, kc in enumerate(pr):
                                contribs.append(
                                    (e_full[pr][:, j * 512 + ts * 128:
                                                j * 512 + (ts + 1) * 128],
                                     kc))
                        if causal:
                            kd = 4 * qb
                            contribs.append(
                                (eA[:, ts * 128:(ts + 1) * 128], kd))
                            if ts >= 1:
                                contribs.append(
                                    (eA[:, 512 + (ts - 1) * 128:
                                         512 + ts * 128], kd + 1))
                            if ts >= 2:
                                contribs.append(
                                    (eB[:, (ts - 2) * 128:(ts - 1) * 128],
                                     kd + 2))
                            if ts >= 3:
                                contribs.append((eB[:, 256:384], kd + 3))
                        for i, (esl, kc) in enumerate(contribs):
                            nc.tensor.matmul(
                                o_ps[:], esl,
                                v_t[b * 4 + kc // 4][:, kc % 4, 0:129],
                                start=(i == 0),
                                stop=(i == len(contribs) - 1))
                        linv = onp.tile([128, 1], F32, name="linv",
                                        tag="linv", bufs=4)
                        nc.vector.reciprocal(linv[:], o_ps[:, 128:129])
                        o_n = onp.tile([128, 128], BF16, name="o_n",
                                       tag="o_n", bufs=12)
                        nc.vector.tensor_scalar(
                            o_n[:], o_ps[:, 0:128], linv[:], None, Alu.mult)
                        o_list.append(o_n)
                    # transpose the PREVIOUS head-group's outputs now: its
                    # DVE normalizes finished while this group's scores ran
                    pending.append((o_list, g, slot, h))
                    if len(pending) > 1:
                        emit_ot(*pending.pop(0))

            def ag_group(g):
                flush_pending()
                nc.gpsimd.collective_compute(
                    "AllGather", Alu.bypass,
                    replica_groups=[list(range(N_CORES))],
                    ins=[ag_in_g[g][:]], outs=[ag_out_g[g][:]])

            # ---------------- emission ------------------------------------
            def make_mtiles(qb):
                if causal:
                    # full blocks only; the diagonal is stair-stepped inline
                    return {}, list(range(4 * qb))
                kcs = list(range(16))
                mtiles = {}
                for kc in range(0, 16, 2):
                    mt = mpool.tile([128, 2, 512], BF16, name="mt",
                                    tag="mt")
                    nc.sync.dma_start(
                        mt[:],
                        maskT_d[kc * 128:(kc + 2) * 128,
                                qb * 512:(qb + 1) * 512]
                        .rearrange("(two p) t -> p two t", p=128))
                    mtiles[(kc, kc + 1)] = mt[:]
                return mtiles, kcs

            with (
                tc.tile_pool(name="wqp", bufs=1) as wqp,
                # bufs=8 = one block: the next block's loads queue behind
                # this block's last readers, which both self-throttles the
                # startup HBM burst and prefetches during attention
                tc.tile_pool(name="xtp", bufs=8) as xtp,
                tc.tile_pool(name="ph1", bufs=2) as ph1,
            ):
                # wq split in two column-half tiles: the Q chains need the
                # first half well before the second, and dependency
                # tracking is tile-granular (loads emitted in proj_block)
                wq_sb = [wqp.tile([128, 32, 256], BF16, name=f"wq_sb{i}")
                         for i in range(2)]
                # tiny dummy AllGather emitted after the weight loads:
                # aligns the cores' start skew while the first projection
                # blocks keep the PE busy, so AG0 isn't delayed by launch
                # offset
                dummy_in = agd.tile([64, 4], BF16, name="dummy_in")
                nc.gpsimd.dma_start(dummy_in[:], cosT_d[0:64, 0:4])
                dummy_out = agd.tile([512, 4], BF16, name="dummy_out",
                                     addr_space="Shared")
                nc.gpsimd.collective_compute(
                    "AllGather", Alu.bypass,
                    replica_groups=[list(range(N_CORES))],
                    ins=[dummy_in[:]], outs=[dummy_out[:]])
                if causal:
                    # causal: chunk (qb, b) needs only blocks <= qb, so
                    # projections and attention interleave per qb
                    for qb in range(4):
                        qts = [proj_block(b * 4 + qb, wq_sb, xtp, ph1,
                                          first=(qb == 0 and b == 0))
                               for b in range(B)]
                        mtiles, kcs = make_mtiles(qb)
                        if qb < 3:
                            for b in range(B):
                                attn_chunk(qb, b, qts[b], mtiles, kcs)
                            ag_group(qb)
                        else:
                            qts3, mtiles3, kcs3 = qts, mtiles, kcs
                else:
                    # general mask: every chunk may attend to every block,
                    # so all projections must complete first
                    allq = [[None, None] for _ in range(4)]
                    for qb in range(4):
                        for b in range(B):
                            allq[qb][b] = proj_block(
                                b * 4 + qb, wq_sb, xtp, ph1,
                                first=(qb == 0 and b == 0))
                    for qb in range(3):
                        mtiles, kcs = make_mtiles(qb)
                        for b in range(B):
                            attn_chunk(qb, b, allq[qb][b], mtiles, kcs)
                        ag_group(qb)
                    qts3 = allq[3]
                    mtiles3, kcs3 = make_mtiles(3)

            # final phase: wo reuses wq's SBUF lifetime; the last two
            # attention chunks are pipelined against the first two
            # out-projections, so every out-proj's transposing og load gates
            # only on already-completed AllGathers
            with (
                tc.tile_pool(name="wop", bufs=1) as wop,
                tc.tile_pool(name="ogp", bufs=2) as ogp,
            ):
                wo_sb = wop.tile([128, 32, QW], BF16, name="wo_sb")
                nc.gpsimd.dma_start(wo_sb[:], wo_d[:])

                def out_proj(ag_out, slot, t0):
                    # one gather per chunk: same descriptor count as two
                    # half loads but double the segment size
                    og_sb = ogp.tile([128, 32, 512], BF16, name="og_sb",
                                     tag="og")
                    nc.sync.dma_start(
                        og_sb[:],
                        ag_out[:, slot * 512:(slot + 1) * 512]
                        .rearrange("(hc p) t -> p hc t", p=128))
                    for tc2 in range(4):
                        op_ps = psum.tile([128, 512], F32, name="op_ps",
                                          tag="ps")
                        for hc in range(32):
                            nc.tensor.matmul(
                                op_ps[:],
                                og_sb[:, hc, tc2 * 128:(tc2 + 1) * 128],
                                wo_sb[:, hc, :],
                                start=(hc == 0), stop=(hc == 31))
                        oo = ph2.tile([128, 512], F32, name="oo",
                                      tag="oo")
                        nc.vector.tensor_copy(oo[:], op_ps[:])
                        nc.scalar.dma_start(
                            out_d[t0 + tc2 * 128: t0 + (tc2 + 1) * 128, :],
                            oo[:])

                for b3 in range(B):
                    attn_chunk(3, b3, qts3[b3], mtiles3, kcs3)
                ag_group(3)
                for qb in range(4):
                    for b in range(B):
                        g, slot = chunk_group(qb, b)
                        out_proj(ag_out_g[g], slot, b * S + qb * 512)

    nc.compile()
    return nc


def _get_nc(causal: bool):
    if causal not in _BUILD_CACHE:
        _BUILD_CACHE[causal] = _build(causal)
    return _BUILD_CACHE[causal]


_DEINT = np.concatenate([np.arange(0, HD, 2), np.arange(1, HD, 2)])


def _deinterleave(w):
    """Permute per-head columns [0,2,..,126,1,3,..,127] (re block | im block)."""
    out = w.copy()
    nh = w.shape[1] // HD
    for h in range(nh):
        out[:, h * HD:(h + 1) * HD] = w[:, h * HD + _DEINT]
    return out


def make_in_maps(x, wq, wk, wv, wo, freqs_cos, freqs_sin, mask, causal):
    bf = ml_dtypes.bfloat16

    def tile_w(w):
        # [4096, W] -> [128, 32, W]: out[p, ic, d] = w[ic*128+p, d]
        return np.ascontiguousarray(
            w.reshape(32, 128, -1).transpose(1, 0, 2)).astype(bf)

    xT = np.asarray(x, np.float32).reshape(TOK, D).T        # [dm, tok]
    # [tb, g, p, ic_in*512+t] = xT[g*512+ic_in*128+p, tb*512+t]
    xt_host = np.ascontiguousarray(
        xT.reshape(8, 4, 128, 8, 512).transpose(3, 0, 2, 1, 4)
        .reshape(8, 8, 128, 2048)).astype(bf)
    cosT = np.ascontiguousarray(np.asarray(freqs_cos, np.float32).T).astype(bf)
    sinT = np.ascontiguousarray(np.asarray(freqs_sin, np.float32).T).astype(bf)
    mask = np.asarray(mask, np.float32)
    # tri01[k, t] = 1 where t >= k (valid) within a diagonal 128-block
    tri01 = np.triu(np.ones((128, 128), np.float32)).astype(bf)
    ident = np.eye(128, dtype=np.float32).astype(bf)

    in_maps = []
    for c in range(N_CORES):
        wqc = tile_w(_deinterleave(
            np.asarray(wq[:, c * QW:(c + 1) * QW], np.float32)))
        m = {
            "xT": xt_host,
            # [128, 32, 512] -> [128, 2, 32, 256] (column halves major)
            "wq": np.ascontiguousarray(
                wqc.reshape(128, 32, 2, 256).transpose(0, 2, 1, 3)),
            "wk": tile_w(_deinterleave(
                np.asarray(wk[:, c * HD:(c + 1) * HD], np.float32))),
            "wv": tile_w(np.asarray(wv[:, c * HD:(c + 1) * HD], np.float32)),
            "wo": tile_w(np.asarray(wo[:, c * QW:(c + 1) * QW], np.float32)),
            "cosT": cosT,
            "sinT": sinT,
            "tri01": tri01,
            "ident": ident,
        }
        if not causal:
            m["maskT"] = np.ascontiguousarray(mask.T).astype(bf)
        in_maps.append(m)
    return in_maps


def _is_causal(mask):
    mask = np.asarray(mask, np.float32)
    expect = np.where(np.tril(np.ones((S, S), bool)), 0.0, NEG).astype(np.float32)
    return np.array_equal(mask, expect)


def kernel(x, wq, wk, wv, wo, cache_k, cache_v, freqs_cos, freqs_sin, mask,
           start_pos):
    assert int(start_pos) == 0, "kernel hardcodes start_pos=0 prefill"
    assert tuple(np.shape(x)) == (B, S, D)
    causal = _is_causal(mask)
    nc = _get_nc(causal)
    in_maps = make_in_maps(x, wq, wk, wv, wo, freqs_cos, freqs_sin, mask,
                           causal)
    res = bass_utils.run_bass_kernel_spmd(
        nc, in_maps, core_ids=list(range(N_CORES)))
    out = np.empty((TOK, D), np.float32)
    for c in range(N_CORES):
        out[:, c * QW:(c + 1) * QW] = res.results[c]["out"]
    return out.reshape(B, S, D)
